# revision 44
# baseline (speedup 1.0000x reference)
"""Grouped-Query Attention block (RMSNorm + RoPE + causal GQA + o_proj) on 8 trn2 NeuronCores.

Sharding: data-parallel over batch (2) x tensor-parallel over kv-head groups (4).
Core c = b*4 + g handles batch b, kv heads {2g, 2g+1}, q heads {8g..8g+7}.
Each core computes a partial o_proj output (T, D) over its 768 head-dims;
host sums the 4 group partials per batch.

v2 layout/schedule (from trace analysis of v1):
  * All matmul operands bf16 (fp32 PSUM accumulate): full PE rate, fast
    weight load, half the DMA bytes.  Norm/rope math stays fp32.
  * RMSNorm 1/rms computed as exp(-0.5*ln(ms+eps)) on the scalar engine:
    ln/exp/copy/square all live in one activation table (no table thrash
    with attention's exp); the broadcast over head rows is a ones-outer-
    product matmul.  No single-lane [1,N] vector reciprocals (4us each in
    v1, 145us total, on the critical path).
  * Per-head even/odd RoPE interleave folded into wq/wk rows (evens 0:48,
    odds 64:112, zero pad), q/k norm weights folded into wq/wk.
  * Attention: scores transposed (k on partitions, q free) in 128-wide
    q chunks with exact tile-level causality; P@V uses P as stationary so
    the output is [q, hd+1] with the softmax denominator as a per-partition
    scalar (vector reciprocal_approx_fast on [128,1]); diagonal masked by a
    constant triangular multiply on the vector engine.
  * Attention outputs transposed back and packed into 6 dense 128-row
    feature tiles so o_proj is 6 (not 8) matmuls per output tile.
  * Issue order software-pipelined: the norm matmuls of output j are issued
    after the projection group of output j+1; attention for head h is
    interleaved between later projection groups; attention finalize for
    chunk qi is issued after the score matmuls of chunk qi+1.
"""

import os
import sys

import numpy as np

sys.path.insert(0, "/opt/trn_rl_repo")

B, T, D = 2, 1024, 3072
NH, NKV, HD = 32, 8, 96
G = 4                 # tensor-parallel groups
QH = NH // G          # q heads per core (8)
KVH = NKV // G        # kv heads per core (2)
NCORES = 8
EPS = 1e-6
SCALE = 1.0 / float(np.sqrt(HD))
KT = D // 128         # 24 contraction tiles over d_model
KTOK = T // 128       # 8 token tiles
NF = QH * HD // 128   # 6 packed feature tiles for o_proj
HALF = T // 2         # 512

_BUILD_CACHE = {}


def _build_nc():
    from contextlib import ExitStack
    from concourse import bacc, tile, mybir

    f32 = mybir.dt.float32
    f32r = mybir.dt.float32r
    bf16 = mybir.dt.bfloat16
    AF = mybir.ActivationFunctionType

    nc = bacc.Bacc("TRN2", target_bir_lowering=False, debug=False,
                   num_devices=NCORES)

    xt_d = nc.dram_tensor("xt", (128, KT, T), bf16, kind="ExternalInput").ap()
    wqt_d = nc.dram_tensor("wqt", (QH, 128, KT, 128), bf16, kind="ExternalInput").ap()
    wkt_d = nc.dram_tensor("wkt", (KVH, 128, KT, 128), bf16, kind="ExternalInput").ap()
    wvt_d = nc.dram_tensor("wvt", (KVH, 128, KT, HD), bf16, kind="ExternalInput").ap()
    wot_d = nc.dram_tensor("wot", (NF, 128, D), bf16, kind="ExternalInput").ap()
    taba_d = nc.dram_tensor("taba", (128, T), f32, kind="ExternalInput").ap()
    tabb_d = nc.dram_tensor("tabb", (128, T), f32, kind="ExternalInput").ap()
    o128_d = nc.dram_tensor("o128", (128, 1), f32r, kind="ExternalInput").ap()
    o1x128_d = nc.dram_tensor("o1x128", (1, 128), f32r, kind="ExternalInput").ap()
    ocol_d = nc.dram_tensor("ocol", (128, KTOK), bf16, kind="ExternalInput").ap()
    ident_d = nc.dram_tensor("ident", (128, 128), bf16, kind="ExternalInput").ap()
    tri_d = nc.dram_tensor("tri", (128, 128), bf16, kind="ExternalInput").ap()
    out_d = nc.dram_tensor("out", (T, D), bf16, kind="ExternalOutput").ap()

    with tile.TileContext(nc) as tc:
        with nc.allow_low_precision(reason="bf16 matmuls, fp32 accumulate"), \
             ExitStack() as ctx:
            const = ctx.enter_context(tc.tile_pool(name="const", bufs=1))
            p_big = ctx.enter_context(tc.tile_pool(name="p_big", bufs=1))

            ident = const.tile([128, 128], bf16, tag="ident")
            nc.sync.dma_start(ident[:], ident_d[:])
            tri = const.tile([128, 128], bf16, tag="tri")
            nc.sync.dma_start(tri[:], tri_d[:])
            eps_t = const.tile([1, 1], f32, tag="eps")
            nc.vector.memset(eps_t[:], EPS)
            ones128 = const.tile([128, 1], f32r, tag="ones128")
            nc.sync.dma_start(ones128[:], o128_d[:])
            o1x128 = const.tile([1, 128], f32r, tag="o1x128")
            nc.sync.dma_start(o1x128[:], o1x128_d[:])
            taba_t = const.tile([128, T], f32, tag="taba")
            tabb_t = const.tile([128, T], f32, tag="tabb")

            # persistent bf16 operands
            qt = [p_big.tile([128, T], bf16, tag=f"qt{h}", name=f"qt{h}")
                  for h in range(QH)]
            ktl = [p_big.tile([128, T], bf16, tag=f"kt{g2}", name=f"kt{g2}")
                   for g2 in range(KVH)]
            vext = [p_big.tile([128, KTOK, HD + 1], bf16, tag=f"vx{g2}",
                               name=f"vx{g2}") for g2 in range(KVH)]
            ao = [p_big.tile([128, T], bf16, tag=f"ao{f}", name=f"ao{f}")
                  for f in range(NF)]
            wot_t = [p_big.tile([128, D], bf16, tag=f"wo{f}", name=f"wo{f}")
                     for f in range(NF)]

            xt_pool = ctx.enter_context(tc.tile_pool(name="xtp", bufs=1))
            xt_t = xt_pool.tile([128, KT, T], bf16, tag="xt")
            # sync carries only the first 3 k-slices so the first projection
            # weight lands early; the rest of x streams on the gpsimd queue
            nc.sync.dma_start(xt_t[:, 0:3, :], xt_d[:, 0:3, :])
            for a, b in [(3, 10), (10, 17), (17, 24)]:
                nc.gpsimd.dma_start(xt_t[:, a:b, :], xt_d[:, a:b, :])

            # zero the rope pad rows (48:64, 112:128 stay zero; rope only
            # writes 0:48 and 64:112)
            for h in range(QH):
                nc.gpsimd.memset(qt[h][:], 0.0)
            for g2 in range(KVH):
                nc.gpsimd.memset(ktl[g2][:], 0.0)
                nc.gpsimd.dma_start(vext[g2][:, :, HD:HD + 1], ocol_d[:])

            # input / weight streaming (rope tables issue inside the loop,
            # after the first projection weight)
            wqk_pool = ctx.enter_context(tc.tile_pool(name="wqk", bufs=2))
            wv_pool = ctx.enter_context(tc.tile_pool(name="wv", bufs=1))

            # SBUF working pools
            sq_pool = ctx.enter_context(tc.tile_pool(name="sqp", bufs=2))
            psc_pool = ctx.enter_context(tc.tile_pool(name="pscp", bufs=3))
            row_pool = ctx.enter_context(tc.tile_pool(name="rowp", bufs=2))
            bcs_pool = ctx.enter_context(tc.tile_pool(name="bcsp", bufs=2))
            qn_pool = ctx.enter_context(tc.tile_pool(name="qnp", bufs=2))
            qsh_pool = ctx.enter_context(tc.tile_pool(name="qshp", bufs=2))
            tm_pool = ctx.enter_context(tc.tile_pool(name="tmp", bufs=4))
            vt_pool = ctx.enter_context(tc.tile_pool(name="vtp", bufs=2))
            pt_pool = ctx.enter_context(tc.tile_pool(name="ptp", bufs=8))
            fin_pool = ctx.enter_context(tc.tile_pool(name="finp", bufs=2))
            ob_pool = ctx.enter_context(tc.tile_pool(name="obp", bufs=2))

            # PSUM: 8 banks of [128, 2KB].  Every buf is a FULL exclusive
            # bank (sub-bank sharing risks concurrent PE-write + engine-read
            # of one bank, which is fatal on HW; full-bank bufs make all
            # reuse WAR-gated by the tile framework).
            ps_pool = ctx.enter_context(
                tc.tile_pool(name="psp", bufs=2, space="PSUM"))   # 2 banks
            ssq_pool = ctx.enter_context(
                tc.tile_pool(name="ssqp", bufs=1, space="PSUM"))  # 1 bank
            bcp_pool = ctx.enter_context(
                tc.tile_pool(name="bcp", bufs=1, space="PSUM"))   # 1 bank
            scp_pool = ctx.enter_context(
                tc.tile_pool(name="scp", bufs=2, space="PSUM"))   # 2 banks
            pop_pool = ctx.enter_context(
                tc.tile_pool(name="pop", bufs=2, space="PSUM"))   # 2 banks

            outs = [("k", 0), ("v", 0), ("k", 1), ("v", 1)] + \
                   [("q", h) for h in range(QH)]

            # ---- per-sub-output stages -------------------------------------
            def stage0(kind, idx, half, ps):
                """scalar square + vector copy of the projection result."""
                if kind == "v":
                    vt = vt_pool.tile([HD, HALF], bf16, tag="vt")
                    nc.scalar.copy(vt[:], ps[0:HD, :])
                    return (vt,)
                sq = sq_pool.tile([128, HALF], f32r, tag="sq")
                nc.scalar.square(sq[:], ps[:])
                psc = psc_pool.tile([128, HALF], f32r, tag="psc")
                nc.vector.tensor_copy(psc[:], ps[:])
                return (sq, psc)

            def stage1(kind, idx, half, st0):
                """tensor: ssq matmul (q/k) or v transposes; scalar: ln/exp."""
                if kind == "v":
                    (vt,) = st0
                    for c in range(HALF // 128):
                        tpt = scp_pool.tile([128, 1024], bf16, tag="sc",
                                            name="tpt")
                        nc.tensor.transpose(
                            tpt[:, 0:HD], vt[:, c * 128:(c + 1) * 128],
                            ident[0:HD, 0:HD])
                        kidx = half * (HALF // 128) + c
                        nc.scalar.copy(vext[idx][:, kidx, 0:HD],
                                       tpt[:, 0:HD])
                    return None
                sq, psc = st0
                ssq = ssq_pool.tile([1, HALF], f32, tag="ssq")
                nc.tensor.matmul(ssq[:], ones128[:], sq[:],
                                 start=True, stop=True)
                rms = row_pool.tile([1, HALF], f32r, tag="rms")
                nc.scalar.activation(rms[:], ssq[:], AF.Sqrt,
                                     bias=eps_t[:], scale=1.0 / HD)
                return (psc, rms)

            def stage2(kind, idx, half, st1):
                """tensor: rms broadcast; vector: reciprocal+normalize+rope."""
                psc, rms = st1
                tsl = slice(half * HALF, (half + 1) * HALF)
                bc = bcp_pool.tile([128, HALF], f32, tag="bc")
                nc.tensor.matmul(bc[:], o1x128[:], rms[:],
                                 start=True, stop=True)
                bcs = bcs_pool.tile([128, HALF], f32, tag="bcs")
                nc.vector.reciprocal_approx_fast(bcs[:], bc[:])
                qn = qn_pool.tile([128, HALF], f32, tag="qn")
                nc.vector.tensor_mul(qn[:], psc[:], bcs[:])
                dst = qt[idx] if kind == "q" else ktl[idx]
                # swap halves (DMA partition shuffle) for partition-aligned rope
                qsh = qsh_pool.tile([128, HALF], f32, tag="qsh")
                nc.gpsimd.dma_start(qsh[0:64, :], qn[64:128, :])
                nc.gpsimd.dma_start(qsh[64:128, :], qn[0:64, :])
                tm1 = tm_pool.tile([128, HALF], f32, tag="tm")
                tm2 = tm_pool.tile([128, HALF], f32, tag="tm")
                nc.vector.tensor_mul(tm1[0:112, :], qn[0:112, :],
                                     taba_t[0:112, tsl])
                nc.vector.tensor_mul(tm2[0:112, :], qsh[0:112, :],
                                     tabb_t[0:112, tsl])
                nc.vector.tensor_add(dst[0:112, tsl], tm1[0:112, :],
                                     tm2[0:112, :])

            # ---- attention -------------------------------------------------
            fin_q = []

            def finalize(h, qi, po):
                """vector: 1/denom + scale; tensor: transpose; pack into ao."""
                r0 = (HD * h) % 128
                f0 = (HD * h) // 128
                n0 = min(HD, 128 - r0)
                qsl = slice(qi * 128, (qi + 1) * 128)
                rinv2 = fin_pool.tile([128, 1], f32, tag="rv2")
                nc.vector.reciprocal_approx_fast(rinv2[:], po[:, HD:HD + 1])
                at_q = fin_pool.tile([128, HD], bf16, tag="atq")
                nc.vector.tensor_scalar_mul(at_q[:], po[:, 0:HD], rinv2[:])

                def emit():
                    tpa = scp_pool.tile([128, 1024], bf16, tag="sc",
                                        name="tpa")
                    nc.tensor.transpose(tpa[0:n0, 0:128], at_q[:, 0:n0],
                                        ident[:, :])
                    stg = fin_pool.tile([128, 128], bf16, tag="stg")
                    nc.vector.tensor_copy(stg[0:n0, :], tpa[0:n0, 0:128])
                    nc.sync.dma_start(ao[f0][r0:r0 + n0, qsl],
                                      stg[0:n0, :])
                    if n0 < HD:
                        n1 = HD - n0
                        tpb = scp_pool.tile([128, 1024], bf16, tag="sc",
                                            name="tpb")
                        nc.tensor.transpose(tpb[0:n1, 0:128], at_q[:, n0:HD],
                                            ident[:, :])
                        stg2 = fin_pool.tile([128, 128], bf16, tag="stg")
                        nc.vector.tensor_copy(stg2[0:n1, :], tpb[0:n1, 0:128])
                        nc.sync.dma_start(ao[f0 + 1][0:n1, qsl],
                                          stg2[0:n1, :])
                fin_q.append(emit)

            def drain_finalize():
                while fin_q:
                    fin_q.pop(0)()

            def attention_chunk(h, qi):
                g2 = h // (QH // KVH)
                qsl = slice(qi * 128, (qi + 1) * 128)
                nk = qi + 1
                # scores in groups of 4 k-tiles per PSUM bank: one 512-wide
                # exp per bank instead of four 128-wide ones
                pts = []
                for b0 in range(0, nk, 4):
                    nb = min(4, nk - b0)
                    sc_t = scp_pool.tile([128, 4, 128], f32,
                                         tag="sc", name="sc_t")
                    for j in range(nb):
                        kt2 = b0 + j
                        nc.tensor.matmul(
                            sc_t[:, j, :],
                            ktl[g2][:, kt2 * 128:(kt2 + 1) * 128],
                            qt[h][:, qsl], start=True, stop=True)
                    pt = pt_pool.tile([128, 4, 128], bf16, tag="pt")
                    nc.scalar.activation(pt[:, 0:nb, :], sc_t[:, 0:nb, :],
                                         AF.Exp, scale=SCALE)
                    pts.append(pt)
                drain_finalize()
                jd = qi % 4
                nc.vector.tensor_mul(pts[-1][:, jd, :], pts[-1][:, jd, :],
                                     tri[:])
                po = pop_pool.tile([128, 512], f32, tag="po")
                for kt2 in range(nk):
                    nc.tensor.matmul(po[:, 0:HD + 1], pts[kt2 // 4][:, kt2 % 4, :],
                                     vext[g2][:, kt2, :],
                                     start=(kt2 == 0), stop=(kt2 == qi))
                finalize(h, qi, po)

            def attention(h):
                for qi in range(KTOK):
                    attention_chunk(h, qi)

            # ---- phase 1+2 interleaved issue loop --------------------------
            SUBS = [(kind, idx, half) for kind, idx in outs
                    for half in range(2)]
            st0q, st1q = [], []
            w_cur = [None]

            for s, (kind, idx, half) in enumerate(SUBS):
                if s == 1:
                    nc.sync.dma_start(taba_t[:], taba_d[:])
                    nc.sync.dma_start(tabb_t[:], tabb_d[:])
                if s == 12:
                    for f in range(NF):
                        nc.sync.dma_start(wot_t[f][:], wot_d[f])
                if half == 0:
                    if kind == "q":
                        w_t = wqk_pool.tile([128, KT, 128], bf16, tag="w")
                        nc.sync.dma_start(w_t[:], wqt_d[idx])
                        mdim = 128
                    elif kind == "k":
                        w_t = wqk_pool.tile([128, KT, 128], bf16, tag="w")
                        nc.sync.dma_start(w_t[:], wkt_d[idx])
                        mdim = 128
                    else:
                        w_t = wv_pool.tile([128, KT, HD], bf16, tag="wv")
                        nc.sync.dma_start(w_t[:], wvt_d[idx])
                        mdim = HD
                    w_cur[0] = (w_t, mdim)
                w_t, mdim = w_cur[0]
                tsl = slice(half * HALF, (half + 1) * HALF)
                ps = ps_pool.tile([128, HALF], f32, tag="ps")
                for kt in range(KT):
                    nc.tensor.matmul(ps[0:mdim, :], w_t[:, kt, :],
                                     xt_t[:, kt, tsl],
                                     start=(kt == 0), stop=(kt == KT - 1))
                # deferred norm work between projection groups
                if st1q:
                    k_, i_, h_, p_ = st1q.pop(0)
                    stage2(k_, i_, h_, p_)
                if st0q:
                    k_, i_, h_, p_ = st0q.pop(0)
                    r = stage1(k_, i_, h_, p_)
                    if r is not None:
                        st1q.append((k_, i_, h_, r))
                st0q.append((kind, idx, half, stage0(kind, idx, half, ps)))

            # flush pipeline, then remaining attention heads
            for _ in range(3):
                if st1q:
                    k_, i_, h_, p_ = st1q.pop(0)
                    stage2(k_, i_, h_, p_)
                if st0q:
                    k_, i_, h_, p_ = st0q.pop(0)
                    r = stage1(k_, i_, h_, p_)
                    if r is not None:
                        st1q.append((k_, i_, h_, r))
            def phase3_col(i):
                isl = slice(i * 128, (i + 1) * 128)
                for jh in range(2):
                    ob = ob_pool.tile([128, D // 2], bf16, tag="ob")
                    for j in range(3):
                        jsl = slice((3 * jh + j) * 512, (3 * jh + j + 1) * 512)
                        ps3 = ps_pool.tile([128, 512], f32, tag="ps")
                        for f in range(NF):
                            nc.tensor.matmul(ps3[:], ao[f][:, isl],
                                             wot_t[f][:, jsl],
                                             start=(f == 0),
                                             stop=(f == NF - 1))
                        nc.scalar.copy(ob[:, j * 512:(j + 1) * 512], ps3[:])
                    nc.sync.dma_start(
                        out_d[isl, jh * (D // 2):(jh + 1) * (D // 2)], ob[:])

            # qi-major attention: all heads compute q-column qi, then the
            # o_proj for column qi-1 interleaves behind it.  Attention's exp
            # work (scalar) hides under o_proj matmuls, and phase 1 ran with
            # zero activation-table switches.
            for qi in range(KTOK):
                for h in range(QH):
                    attention_chunk(h, qi)
                if qi >= 1:
                    phase3_col(qi - 1)
            drain_finalize()
            phase3_col(KTOK - 1)

    nc.compile()
    return nc


def get_nc():
    if "nc" not in _BUILD_CACHE:
        _BUILD_CACHE["nc"] = _build_nc()
    return _BUILD_CACHE["nc"]


def _permpad_rows(w96):
    """(96, N) head rows -> (128, N): evens at 0:48, odds at 64:112, pad 0."""
    out = np.zeros((128, w96.shape[1]), np.float32)
    out[0:48] = w96[0::2]
    out[64:112] = w96[1::2]
    return out


def _lhsT_tiles(wT, m):
    """(D, m) -> (128, KT, m) lhsT tile layout (contraction on partitions)."""
    return np.ascontiguousarray(
        wT.reshape(KT, 128, m).transpose(1, 0, 2)).astype(np.float32)


def prepare_in_maps(x, wq, wk, wv, wo, q_norm_w, k_norm_w, cos, sin):
    import ml_dtypes
    bf16 = ml_dtypes.bfloat16

    x = np.asarray(x, np.float32)
    wq = np.asarray(wq, np.float32) * np.tile(
        np.asarray(q_norm_w, np.float32), NH)[:, None]
    wk = np.asarray(wk, np.float32) * np.tile(
        np.asarray(k_norm_w, np.float32), NKV)[:, None]
    wv = np.asarray(wv, np.float32)
    wo = np.asarray(wo, np.float32)
    cos = np.asarray(cos, np.float32)
    sin = np.asarray(sin, np.float32)

    # fused-rope tables: dst = qn*P + qsh*Q (P=cos rows, Q=sin rows with
    # the even-row sign folded in); pad rows stay zero
    taba = np.zeros((128, T), np.float32)
    tabb = np.zeros((128, T), np.float32)
    taba[0:48] = cos[:, 0::2].T
    taba[64:112] = cos[:, 1::2].T
    tabb[0:48] = -sin[:, 0::2].T
    tabb[64:112] = sin[:, 1::2].T

    xts = []
    for b in range(B):
        xT = np.ascontiguousarray(x[b].T)  # (D, T)
        xts.append(np.ascontiguousarray(
            xT.reshape(KT, 128, T).transpose(1, 0, 2)).astype(bf16))

    tri = np.triu(np.ones((128, 128), np.float32)).astype(bf16)
    identm = np.eye(128, dtype=np.float32).astype(bf16)

    in_maps = []
    for c in range(NCORES):
        b, g = divmod(c, G)
        wqt = np.stack([
            _lhsT_tiles(_permpad_rows(
                wq[(g * QH + i) * HD:(g * QH + i + 1) * HD]).T, 128)
            for i in range(QH)]).astype(bf16)
        wkt = np.stack([
            _lhsT_tiles(_permpad_rows(
                wk[(g * KVH + i) * HD:(g * KVH + i + 1) * HD]).T, 128)
            for i in range(KVH)]).astype(bf16)
        wvt = np.stack([
            _lhsT_tiles(np.ascontiguousarray(
                wv[(g * KVH + i) * HD:(g * KVH + i + 1) * HD].T), HD)
            for i in range(KVH)]).astype(bf16)
        wo_shT = np.ascontiguousarray(
            wo[:, g * QH * HD:(g + 1) * QH * HD].T)  # (768, D)
        wot = np.ascontiguousarray(
            wo_shT.reshape(NF, 128, D)).astype(bf16)
        in_maps.append({
            "xt": xts[b], "wqt": wqt, "wkt": wkt, "wvt": wvt, "wot": wot,
            "taba": taba, "tabb": tabb,
            "o128": np.ones((128, 1), np.float32),
            "o1x128": np.ones((1, 128), np.float32),
            "ocol": np.ones((128, KTOK), np.float32).astype(bf16),
            "ident": identm, "tri": tri,
        })
    return in_maps


def kernel(**inputs):
    from concourse import bass_utils

    nc = get_nc()
    in_maps = prepare_in_maps(
        inputs["x"], inputs["wq"], inputs["wk"], inputs["wv"], inputs["wo"],
        inputs["q_norm_w"], inputs["k_norm_w"], inputs["cos"], inputs["sin"])
    trace = bool(int(os.environ.get("BASS_KERNEL_TRACE", "0")))
    tmpdir = os.environ.get("BASS_KERNEL_TMPDIR") or None
    res = bass_utils.run_bass_kernel_spmd(
        nc, in_maps, core_ids=list(range(NCORES)), trace=trace, tmpdir=tmpdir)
    _BUILD_CACHE["last_result"] = res
    partials = [np.asarray(r["out"]).astype(np.float32) for r in res.results]
    out = np.empty((B, T, D), np.float32)
    for b in range(B):
        out[b] = np.sum(np.stack(partials[b * G:(b + 1) * G]), axis=0,
                        dtype=np.float64).astype(np.float32)
    return out


# revision 46
# speedup vs baseline: 1.0059x; 1.0059x over previous
"""Grouped-Query Attention block (RMSNorm + RoPE + causal GQA + o_proj) on 8 trn2 NeuronCores.

Sharding: data-parallel over batch (2) x tensor-parallel over kv-head groups (4).
Core c = b*4 + g handles batch b, kv heads {2g, 2g+1}, q heads {8g..8g+7}.
Each core computes a partial o_proj output (T, D) over its 768 head-dims;
host sums the 4 group partials per batch.

v2 layout/schedule (from trace analysis of v1):
  * All matmul operands bf16 (fp32 PSUM accumulate): full PE rate, fast
    weight load, half the DMA bytes.  Norm/rope math stays fp32.
  * RMSNorm 1/rms computed as exp(-0.5*ln(ms+eps)) on the scalar engine:
    ln/exp/copy/square all live in one activation table (no table thrash
    with attention's exp); the broadcast over head rows is a ones-outer-
    product matmul.  No single-lane [1,N] vector reciprocals (4us each in
    v1, 145us total, on the critical path).
  * Per-head even/odd RoPE interleave folded into wq/wk rows (evens 0:48,
    odds 64:112, zero pad), q/k norm weights folded into wq/wk.
  * Attention: scores transposed (k on partitions, q free) in 128-wide
    q chunks with exact tile-level causality; P@V uses P as stationary so
    the output is [q, hd+1] with the softmax denominator as a per-partition
    scalar (vector reciprocal_approx_fast on [128,1]); diagonal masked by a
    constant triangular multiply on the vector engine.
  * Attention outputs transposed back and packed into 6 dense 128-row
    feature tiles so o_proj is 6 (not 8) matmuls per output tile.
  * Issue order software-pipelined: the norm matmuls of output j are issued
    after the projection group of output j+1; attention for head h is
    interleaved between later projection groups; attention finalize for
    chunk qi is issued after the score matmuls of chunk qi+1.
"""

import os
import sys

import numpy as np

sys.path.insert(0, "/opt/trn_rl_repo")

B, T, D = 2, 1024, 3072
NH, NKV, HD = 32, 8, 96
G = 4                 # tensor-parallel groups
QH = NH // G          # q heads per core (8)
KVH = NKV // G        # kv heads per core (2)
NCORES = 8
EPS = 1e-6
SCALE = 1.0 / float(np.sqrt(HD))
KT = D // 128         # 24 contraction tiles over d_model
KTOK = T // 128       # 8 token tiles
NF = QH * HD // 128   # 6 packed feature tiles for o_proj
HALF = T // 2         # 512

_BUILD_CACHE = {}


def _build_nc():
    from contextlib import ExitStack
    from concourse import bacc, tile, mybir

    f32 = mybir.dt.float32
    f32r = mybir.dt.float32r
    bf16 = mybir.dt.bfloat16
    AF = mybir.ActivationFunctionType

    nc = bacc.Bacc("TRN2", target_bir_lowering=False, debug=False,
                   num_devices=NCORES)

    xt_d = nc.dram_tensor("xt", (128, KT, T), bf16, kind="ExternalInput").ap()
    wqt_d = nc.dram_tensor("wqt", (QH, 128, KT, 128), bf16, kind="ExternalInput").ap()
    wkt_d = nc.dram_tensor("wkt", (KVH, 128, KT, 128), bf16, kind="ExternalInput").ap()
    wvt_d = nc.dram_tensor("wvt", (KVH, 128, KT, HD), bf16, kind="ExternalInput").ap()
    wot_d = nc.dram_tensor("wot", (NF, 128, D), bf16, kind="ExternalInput").ap()
    taba_d = nc.dram_tensor("taba", (128, T), f32, kind="ExternalInput").ap()
    tabb_d = nc.dram_tensor("tabb", (128, T), f32, kind="ExternalInput").ap()
    o128_d = nc.dram_tensor("o128", (128, 1), f32r, kind="ExternalInput").ap()
    o1x128_d = nc.dram_tensor("o1x128", (1, 128), f32r, kind="ExternalInput").ap()
    ocol_d = nc.dram_tensor("ocol", (128, KTOK), bf16, kind="ExternalInput").ap()
    ident_d = nc.dram_tensor("ident", (128, 128), bf16, kind="ExternalInput").ap()
    tri_d = nc.dram_tensor("tri", (128, 128), bf16, kind="ExternalInput").ap()
    out_d = nc.dram_tensor("out", (T, D), bf16, kind="ExternalOutput").ap()

    with tile.TileContext(nc) as tc:
        with nc.allow_low_precision(reason="bf16 matmuls, fp32 accumulate"), \
             ExitStack() as ctx:
            const = ctx.enter_context(tc.tile_pool(name="const", bufs=1))
            p_big = ctx.enter_context(tc.tile_pool(name="p_big", bufs=1))

            ident = const.tile([128, 128], bf16, tag="ident")
            nc.sync.dma_start(ident[:], ident_d[:])
            tri = const.tile([128, 128], bf16, tag="tri")
            nc.sync.dma_start(tri[:], tri_d[:])
            eps_t = const.tile([1, 1], f32, tag="eps")
            nc.vector.memset(eps_t[:], EPS)
            ones128 = const.tile([128, 1], f32r, tag="ones128")
            nc.sync.dma_start(ones128[:], o128_d[:])
            o1x128 = const.tile([1, 128], f32r, tag="o1x128")
            nc.sync.dma_start(o1x128[:], o1x128_d[:])
            taba_t = const.tile([128, T], f32, tag="taba")
            tabb_t = const.tile([128, T], f32, tag="tabb")

            # persistent bf16 operands
            qt = [p_big.tile([128, T], bf16, tag=f"qt{h}", name=f"qt{h}")
                  for h in range(QH)]
            ktl = [p_big.tile([128, T], bf16, tag=f"kt{g2}", name=f"kt{g2}")
                   for g2 in range(KVH)]
            vext = [p_big.tile([128, KTOK, HD + 1], bf16, tag=f"vx{g2}",
                               name=f"vx{g2}") for g2 in range(KVH)]
            ao = [p_big.tile([128, T], bf16, tag=f"ao{f}", name=f"ao{f}")
                  for f in range(NF)]
            wot_t = [p_big.tile([128, D], bf16, tag=f"wo{f}", name=f"wo{f}")
                     for f in range(NF)]

            xt_pool = ctx.enter_context(tc.tile_pool(name="xtp", bufs=1))
            xt_t = xt_pool.tile([128, KT, T], bf16, tag="xt")
            nc.sync.dma_start(xt_t[:, 0:6, :], xt_d[:, 0:6, :])
            for c in range(1, 4):
                nc.gpsimd.dma_start(xt_t[:, 6 * c:6 * c + 6, :],
                                    xt_d[:, 6 * c:6 * c + 6, :])

            # zero the rope pad rows (48:64, 112:128 stay zero; rope only
            # writes 0:48 and 64:112)
            for h in range(QH):
                nc.gpsimd.memset(qt[h][:], 0.0)
            for g2 in range(KVH):
                nc.gpsimd.memset(ktl[g2][:], 0.0)
                nc.gpsimd.dma_start(vext[g2][:, :, HD:HD + 1], ocol_d[:])

            # input / weight streaming
            nc.sync.dma_start(taba_t[:], taba_d[:])
            nc.sync.dma_start(tabb_t[:], tabb_d[:])
            wqk_pool = ctx.enter_context(tc.tile_pool(name="wqk", bufs=2))
            wv_pool = ctx.enter_context(tc.tile_pool(name="wv", bufs=1))

            # SBUF working pools
            sq_pool = ctx.enter_context(tc.tile_pool(name="sqp", bufs=2))
            psc_pool = ctx.enter_context(tc.tile_pool(name="pscp", bufs=3))
            row_pool = ctx.enter_context(tc.tile_pool(name="rowp", bufs=2))
            bcs_pool = ctx.enter_context(tc.tile_pool(name="bcsp", bufs=2))
            qn_pool = ctx.enter_context(tc.tile_pool(name="qnp", bufs=2))
            qsh_pool = ctx.enter_context(tc.tile_pool(name="qshp", bufs=2))
            tm_pool = ctx.enter_context(tc.tile_pool(name="tmp", bufs=4))
            vt_pool = ctx.enter_context(tc.tile_pool(name="vtp", bufs=2))
            pt_pool = ctx.enter_context(tc.tile_pool(name="ptp", bufs=8))
            fin_pool = ctx.enter_context(tc.tile_pool(name="finp", bufs=2))
            ob_pool = ctx.enter_context(tc.tile_pool(name="obp", bufs=2))

            # PSUM: 8 banks of [128, 2KB].  Every buf is a FULL exclusive
            # bank (sub-bank sharing risks concurrent PE-write + engine-read
            # of one bank, which is fatal on HW; full-bank bufs make all
            # reuse WAR-gated by the tile framework).
            ps_pool = ctx.enter_context(
                tc.tile_pool(name="psp", bufs=2, space="PSUM"))   # 2 banks
            ssq_pool = ctx.enter_context(
                tc.tile_pool(name="ssqp", bufs=1, space="PSUM"))  # 1 bank
            bcp_pool = ctx.enter_context(
                tc.tile_pool(name="bcp", bufs=1, space="PSUM"))   # 1 bank
            scp_pool = ctx.enter_context(
                tc.tile_pool(name="scp", bufs=2, space="PSUM"))   # 2 banks
            pop_pool = ctx.enter_context(
                tc.tile_pool(name="pop", bufs=2, space="PSUM"))   # 2 banks

            outs = [("k", 0), ("v", 0), ("k", 1), ("v", 1)] + \
                   [("q", h) for h in range(QH)]

            # ---- per-sub-output stages -------------------------------------
            def stage0(kind, idx, half, ps):
                """scalar square + vector copy of the projection result."""
                if kind == "v":
                    vt = vt_pool.tile([HD, HALF], bf16, tag="vt")
                    nc.scalar.copy(vt[:], ps[0:HD, :])
                    return (vt,)
                sq = sq_pool.tile([128, HALF], f32r, tag="sq")
                nc.scalar.square(sq[:], ps[:])
                psc = psc_pool.tile([128, HALF], f32r, tag="psc")
                nc.vector.tensor_copy(psc[:], ps[:])
                return (sq, psc)

            def stage1(kind, idx, half, st0):
                """tensor: ssq matmul (q/k) or v transposes; scalar: ln/exp."""
                if kind == "v":
                    (vt,) = st0
                    for c in range(HALF // 128):
                        tpt = scp_pool.tile([128, 1024], bf16, tag="sc",
                                            name="tpt")
                        nc.tensor.transpose(
                            tpt[:, 0:HD], vt[:, c * 128:(c + 1) * 128],
                            ident[0:HD, 0:HD])
                        kidx = half * (HALF // 128) + c
                        nc.scalar.copy(vext[idx][:, kidx, 0:HD],
                                       tpt[:, 0:HD])
                    return None
                sq, psc = st0
                ssq = ssq_pool.tile([1, HALF], f32, tag="ssq")
                nc.tensor.matmul(ssq[:], ones128[:], sq[:],
                                 start=True, stop=True)
                rms = row_pool.tile([1, HALF], f32r, tag="rms")
                nc.scalar.activation(rms[:], ssq[:], AF.Sqrt,
                                     bias=eps_t[:], scale=1.0 / HD)
                return (psc, rms)

            def stage2(kind, idx, half, st1):
                """tensor: rms broadcast; vector: reciprocal+normalize+rope."""
                psc, rms = st1
                tsl = slice(half * HALF, (half + 1) * HALF)
                bc = bcp_pool.tile([128, HALF], f32, tag="bc")
                nc.tensor.matmul(bc[:], o1x128[:], rms[:],
                                 start=True, stop=True)
                bcs = bcs_pool.tile([128, HALF], f32, tag="bcs")
                nc.vector.reciprocal_approx_fast(bcs[:], bc[:])
                qn = qn_pool.tile([128, HALF], f32, tag="qn")
                nc.vector.tensor_mul(qn[:], psc[:], bcs[:])
                dst = qt[idx] if kind == "q" else ktl[idx]
                # swap halves (DMA partition shuffle) for partition-aligned rope
                qsh = qsh_pool.tile([128, HALF], f32, tag="qsh")
                nc.gpsimd.dma_start(qsh[0:64, :], qn[64:128, :])
                nc.gpsimd.dma_start(qsh[64:128, :], qn[0:64, :])
                tm1 = tm_pool.tile([128, HALF], f32, tag="tm")
                tm2 = tm_pool.tile([128, HALF], f32, tag="tm")
                nc.vector.tensor_mul(tm1[0:112, :], qn[0:112, :],
                                     taba_t[0:112, tsl])
                nc.vector.tensor_mul(tm2[0:112, :], qsh[0:112, :],
                                     tabb_t[0:112, tsl])
                nc.vector.tensor_add(dst[0:112, tsl], tm1[0:112, :],
                                     tm2[0:112, :])

            # ---- attention -------------------------------------------------
            fin_q = []

            def finalize(h, qi, po):
                """vector: 1/denom + scale; tensor: transpose; pack into ao."""
                r0 = (HD * h) % 128
                f0 = (HD * h) // 128
                n0 = min(HD, 128 - r0)
                qsl = slice(qi * 128, (qi + 1) * 128)
                rinv2 = fin_pool.tile([128, 1], f32, tag="rv2")
                nc.vector.reciprocal_approx_fast(rinv2[:], po[:, HD:HD + 1])
                at_q = fin_pool.tile([128, HD], bf16, tag="atq")
                nc.vector.tensor_scalar_mul(at_q[:], po[:, 0:HD], rinv2[:])

                def emit():
                    tpa = scp_pool.tile([128, 1024], bf16, tag="sc",
                                        name="tpa")
                    nc.tensor.transpose(tpa[0:n0, 0:128], at_q[:, 0:n0],
                                        ident[:, :])
                    stg = fin_pool.tile([128, 128], bf16, tag="stg")
                    nc.vector.tensor_copy(stg[0:n0, :], tpa[0:n0, 0:128])
                    nc.sync.dma_start(ao[f0][r0:r0 + n0, qsl],
                                      stg[0:n0, :])
                    if n0 < HD:
                        n1 = HD - n0
                        tpb = scp_pool.tile([128, 1024], bf16, tag="sc",
                                            name="tpb")
                        nc.tensor.transpose(tpb[0:n1, 0:128], at_q[:, n0:HD],
                                            ident[:, :])
                        stg2 = fin_pool.tile([128, 128], bf16, tag="stg")
                        nc.vector.tensor_copy(stg2[0:n1, :], tpb[0:n1, 0:128])
                        nc.sync.dma_start(ao[f0 + 1][0:n1, qsl],
                                          stg2[0:n1, :])
                fin_q.append(emit)

            def drain_finalize():
                while fin_q:
                    fin_q.pop(0)()

            def attention_chunk(h, qi):
                g2 = h // (QH // KVH)
                qsl = slice(qi * 128, (qi + 1) * 128)
                nk = qi + 1
                # scores in groups of 4 k-tiles per PSUM bank: one 512-wide
                # exp per bank instead of four 128-wide ones
                pts = []
                for b0 in range(0, nk, 4):
                    nb = min(4, nk - b0)
                    sc_t = scp_pool.tile([128, 4, 128], f32,
                                         tag="sc", name="sc_t")
                    for j in range(nb):
                        kt2 = b0 + j
                        nc.tensor.matmul(
                            sc_t[:, j, :],
                            ktl[g2][:, kt2 * 128:(kt2 + 1) * 128],
                            qt[h][:, qsl], start=True, stop=True)
                    pt = pt_pool.tile([128, 4, 128], bf16, tag="pt")
                    nc.scalar.activation(pt[:, 0:nb, :], sc_t[:, 0:nb, :],
                                         AF.Exp, scale=SCALE)
                    pts.append(pt)
                drain_finalize()
                jd = qi % 4
                nc.vector.tensor_mul(pts[-1][:, jd, :], pts[-1][:, jd, :],
                                     tri[:])
                po = pop_pool.tile([128, 512], f32, tag="po")
                for kt2 in range(nk):
                    nc.tensor.matmul(po[:, 0:HD + 1], pts[kt2 // 4][:, kt2 % 4, :],
                                     vext[g2][:, kt2, :],
                                     start=(kt2 == 0), stop=(kt2 == qi))
                finalize(h, qi, po)

            def attention(h):
                for qi in range(KTOK):
                    attention_chunk(h, qi)

            # ---- phase 1+2 interleaved issue loop --------------------------
            SUBS = [(kind, idx, half) for kind, idx in outs
                    for half in range(2)]
            st0q, st1q = [], []
            w_cur = [None]

            for s, (kind, idx, half) in enumerate(SUBS):
                if s == 12:
                    for f in range(NF):
                        nc.sync.dma_start(wot_t[f][:], wot_d[f])
                if half == 0:
                    if kind == "q":
                        w_t = wqk_pool.tile([128, KT, 128], bf16, tag="w")
                        nc.sync.dma_start(w_t[:], wqt_d[idx])
                        mdim = 128
                    elif kind == "k":
                        w_t = wqk_pool.tile([128, KT, 128], bf16, tag="w")
                        nc.sync.dma_start(w_t[:], wkt_d[idx])
                        mdim = 128
                    else:
                        w_t = wv_pool.tile([128, KT, HD], bf16, tag="wv")
                        nc.sync.dma_start(w_t[:], wvt_d[idx])
                        mdim = HD
                    w_cur[0] = (w_t, mdim)
                w_t, mdim = w_cur[0]
                tsl = slice(half * HALF, (half + 1) * HALF)
                ps = ps_pool.tile([128, HALF], f32, tag="ps")
                for kt in range(KT):
                    nc.tensor.matmul(ps[0:mdim, :], w_t[:, kt, :],
                                     xt_t[:, kt, tsl],
                                     start=(kt == 0), stop=(kt == KT - 1))
                # deferred norm work between projection groups
                if st1q:
                    k_, i_, h_, p_ = st1q.pop(0)
                    stage2(k_, i_, h_, p_)
                if st0q:
                    k_, i_, h_, p_ = st0q.pop(0)
                    r = stage1(k_, i_, h_, p_)
                    if r is not None:
                        st1q.append((k_, i_, h_, r))
                st0q.append((kind, idx, half, stage0(kind, idx, half, ps)))

            # flush pipeline, then remaining attention heads
            for _ in range(3):
                if st1q:
                    k_, i_, h_, p_ = st1q.pop(0)
                    stage2(k_, i_, h_, p_)
                if st0q:
                    k_, i_, h_, p_ = st0q.pop(0)
                    r = stage1(k_, i_, h_, p_)
                    if r is not None:
                        st1q.append((k_, i_, h_, r))
            def phase3_col(i):
                isl = slice(i * 128, (i + 1) * 128)
                for jh in range(2):
                    ob = ob_pool.tile([128, D // 2], bf16, tag="ob")
                    for j in range(3):
                        jsl = slice((3 * jh + j) * 512, (3 * jh + j + 1) * 512)
                        ps3 = ps_pool.tile([128, 512], f32, tag="ps")
                        for f in range(NF):
                            nc.tensor.matmul(ps3[:], ao[f][:, isl],
                                             wot_t[f][:, jsl],
                                             start=(f == 0),
                                             stop=(f == NF - 1))
                        nc.scalar.copy(ob[:, j * 512:(j + 1) * 512], ps3[:])
                    # gpsimd queue: keeps the big output transfers from
                    # head-of-line blocking the latency-critical ao packs
                    # on the sync queue
                    nc.gpsimd.dma_start(
                        out_d[isl, jh * (D // 2):(jh + 1) * (D // 2)], ob[:])

            # qi-major attention: all heads compute q-column qi, then the
            # o_proj for column qi-1 interleaves behind it.  Attention's exp
            # work (scalar) hides under o_proj matmuls, and phase 1 ran with
            # zero activation-table switches.
            for qi in range(KTOK):
                for h in range(QH):
                    attention_chunk(h, qi)
                if qi >= 1:
                    phase3_col(qi - 1)
            drain_finalize()
            phase3_col(KTOK - 1)

    nc.compile()
    return nc


def get_nc():
    if "nc" not in _BUILD_CACHE:
        _BUILD_CACHE["nc"] = _build_nc()
    return _BUILD_CACHE["nc"]


def _permpad_rows(w96):
    """(96, N) head rows -> (128, N): evens at 0:48, odds at 64:112, pad 0."""
    out = np.zeros((128, w96.shape[1]), np.float32)
    out[0:48] = w96[0::2]
    out[64:112] = w96[1::2]
    return out


def _lhsT_tiles(wT, m):
    """(D, m) -> (128, KT, m) lhsT tile layout (contraction on partitions)."""
    return np.ascontiguousarray(
        wT.reshape(KT, 128, m).transpose(1, 0, 2)).astype(np.float32)


def prepare_in_maps(x, wq, wk, wv, wo, q_norm_w, k_norm_w, cos, sin):
    import ml_dtypes
    bf16 = ml_dtypes.bfloat16

    x = np.asarray(x, np.float32)
    wq = np.asarray(wq, np.float32) * np.tile(
        np.asarray(q_norm_w, np.float32), NH)[:, None]
    wk = np.asarray(wk, np.float32) * np.tile(
        np.asarray(k_norm_w, np.float32), NKV)[:, None]
    wv = np.asarray(wv, np.float32)
    wo = np.asarray(wo, np.float32)
    cos = np.asarray(cos, np.float32)
    sin = np.asarray(sin, np.float32)

    # fused-rope tables: dst = qn*P + qsh*Q (P=cos rows, Q=sin rows with
    # the even-row sign folded in); pad rows stay zero
    taba = np.zeros((128, T), np.float32)
    tabb = np.zeros((128, T), np.float32)
    taba[0:48] = cos[:, 0::2].T
    taba[64:112] = cos[:, 1::2].T
    tabb[0:48] = -sin[:, 0::2].T
    tabb[64:112] = sin[:, 1::2].T

    xts = []
    for b in range(B):
        xT = np.ascontiguousarray(x[b].T)  # (D, T)
        xts.append(np.ascontiguousarray(
            xT.reshape(KT, 128, T).transpose(1, 0, 2)).astype(bf16))

    tri = np.triu(np.ones((128, 128), np.float32)).astype(bf16)
    identm = np.eye(128, dtype=np.float32).astype(bf16)

    in_maps = []
    for c in range(NCORES):
        b, g = divmod(c, G)
        wqt = np.stack([
            _lhsT_tiles(_permpad_rows(
                wq[(g * QH + i) * HD:(g * QH + i + 1) * HD]).T, 128)
            for i in range(QH)]).astype(bf16)
        wkt = np.stack([
            _lhsT_tiles(_permpad_rows(
                wk[(g * KVH + i) * HD:(g * KVH + i + 1) * HD]).T, 128)
            for i in range(KVH)]).astype(bf16)
        wvt = np.stack([
            _lhsT_tiles(np.ascontiguousarray(
                wv[(g * KVH + i) * HD:(g * KVH + i + 1) * HD].T), HD)
            for i in range(KVH)]).astype(bf16)
        wo_shT = np.ascontiguousarray(
            wo[:, g * QH * HD:(g + 1) * QH * HD].T)  # (768, D)
        wot = np.ascontiguousarray(
            wo_shT.reshape(NF, 128, D)).astype(bf16)
        in_maps.append({
            "xt": xts[b], "wqt": wqt, "wkt": wkt, "wvt": wvt, "wot": wot,
            "taba": taba, "tabb": tabb,
            "o128": np.ones((128, 1), np.float32),
            "o1x128": np.ones((1, 128), np.float32),
            "ocol": np.ones((128, KTOK), np.float32).astype(bf16),
            "ident": identm, "tri": tri,
        })
    return in_maps


def kernel(**inputs):
    from concourse import bass_utils

    nc = get_nc()
    in_maps = prepare_in_maps(
        inputs["x"], inputs["wq"], inputs["wk"], inputs["wv"], inputs["wo"],
        inputs["q_norm_w"], inputs["k_norm_w"], inputs["cos"], inputs["sin"])
    trace = bool(int(os.environ.get("BASS_KERNEL_TRACE", "0")))
    tmpdir = os.environ.get("BASS_KERNEL_TMPDIR") or None
    res = bass_utils.run_bass_kernel_spmd(
        nc, in_maps, core_ids=list(range(NCORES)), trace=trace, tmpdir=tmpdir)
    _BUILD_CACHE["last_result"] = res
    partials = [np.asarray(r["out"]).astype(np.float32) for r in res.results]
    out = np.empty((B, T, D), np.float32)
    for b in range(B):
        out[b] = np.sum(np.stack(partials[b * G:(b + 1) * G]), axis=0,
                        dtype=np.float64).astype(np.float32)
    return out


# revision 48
# speedup vs baseline: 1.0114x; 1.0054x over previous
"""Grouped-Query Attention block (RMSNorm + RoPE + causal GQA + o_proj) on 8 trn2 NeuronCores.

Sharding: data-parallel over batch (2) x tensor-parallel over kv-head groups (4).
Core c = b*4 + g handles batch b, kv heads {2g, 2g+1}, q heads {8g..8g+7}.
Each core computes a partial o_proj output (T, D) over its 768 head-dims;
host sums the 4 group partials per batch.

v2 layout/schedule (from trace analysis of v1):
  * All matmul operands bf16 (fp32 PSUM accumulate): full PE rate, fast
    weight load, half the DMA bytes.  Norm/rope math stays fp32.
  * RMSNorm 1/rms computed as exp(-0.5*ln(ms+eps)) on the scalar engine:
    ln/exp/copy/square all live in one activation table (no table thrash
    with attention's exp); the broadcast over head rows is a ones-outer-
    product matmul.  No single-lane [1,N] vector reciprocals (4us each in
    v1, 145us total, on the critical path).
  * Per-head even/odd RoPE interleave folded into wq/wk rows (evens 0:48,
    odds 64:112, zero pad), q/k norm weights folded into wq/wk.
  * Attention: scores transposed (k on partitions, q free) in 128-wide
    q chunks with exact tile-level causality; P@V uses P as stationary so
    the output is [q, hd+1] with the softmax denominator as a per-partition
    scalar (vector reciprocal_approx_fast on [128,1]); diagonal masked by a
    constant triangular multiply on the vector engine.
  * Attention outputs transposed back and packed into 6 dense 128-row
    feature tiles so o_proj is 6 (not 8) matmuls per output tile.
  * Issue order software-pipelined: the norm matmuls of output j are issued
    after the projection group of output j+1; attention for head h is
    interleaved between later projection groups; attention finalize for
    chunk qi is issued after the score matmuls of chunk qi+1.
"""

import os
import sys

import numpy as np

sys.path.insert(0, "/opt/trn_rl_repo")

B, T, D = 2, 1024, 3072
NH, NKV, HD = 32, 8, 96
G = 4                 # tensor-parallel groups
QH = NH // G          # q heads per core (8)
KVH = NKV // G        # kv heads per core (2)
NCORES = 8
EPS = 1e-6
SCALE = 1.0 / float(np.sqrt(HD))
KT = D // 128         # 24 contraction tiles over d_model
KTOK = T // 128       # 8 token tiles
NF = QH * HD // 128   # 6 packed feature tiles for o_proj
HALF = T // 2         # 512

_BUILD_CACHE = {}


def _build_nc():
    from contextlib import ExitStack
    from concourse import bacc, tile, mybir

    f32 = mybir.dt.float32
    f32r = mybir.dt.float32r
    bf16 = mybir.dt.bfloat16
    AF = mybir.ActivationFunctionType

    nc = bacc.Bacc("TRN2", target_bir_lowering=False, debug=False,
                   num_devices=NCORES)

    xt_d = nc.dram_tensor("xt", (128, KT, T), bf16, kind="ExternalInput").ap()
    wqt_d = nc.dram_tensor("wqt", (QH, 128, KT, 128), bf16, kind="ExternalInput").ap()
    wkt_d = nc.dram_tensor("wkt", (KVH, 128, KT, 128), bf16, kind="ExternalInput").ap()
    wvt_d = nc.dram_tensor("wvt", (KVH, 128, KT, HD), bf16, kind="ExternalInput").ap()
    wot_d = nc.dram_tensor("wot", (NF, 128, D), bf16, kind="ExternalInput").ap()
    taba_d = nc.dram_tensor("taba", (128, T), f32, kind="ExternalInput").ap()
    tabb_d = nc.dram_tensor("tabb", (128, T), f32, kind="ExternalInput").ap()
    o128_d = nc.dram_tensor("o128", (128, 1), f32r, kind="ExternalInput").ap()
    o1x128_d = nc.dram_tensor("o1x128", (1, 128), f32r, kind="ExternalInput").ap()
    ocol_d = nc.dram_tensor("ocol", (128, KTOK), bf16, kind="ExternalInput").ap()
    ident_d = nc.dram_tensor("ident", (128, 128), bf16, kind="ExternalInput").ap()
    tri_d = nc.dram_tensor("tri", (128, 128), bf16, kind="ExternalInput").ap()
    out_d = nc.dram_tensor("out", (T, D), bf16, kind="ExternalOutput").ap()

    with tile.TileContext(nc) as tc:
        with nc.allow_low_precision(reason="bf16 matmuls, fp32 accumulate"), \
             ExitStack() as ctx:
            const = ctx.enter_context(tc.tile_pool(name="const", bufs=1))
            p_big = ctx.enter_context(tc.tile_pool(name="p_big", bufs=1))

            ident = const.tile([128, 128], bf16, tag="ident")
            nc.sync.dma_start(ident[:], ident_d[:])
            tri = const.tile([128, 128], bf16, tag="tri")
            nc.sync.dma_start(tri[:], tri_d[:])
            eps_t = const.tile([1, 1], f32, tag="eps")
            nc.vector.memset(eps_t[:], EPS)
            ones128 = const.tile([128, 1], f32r, tag="ones128")
            nc.sync.dma_start(ones128[:], o128_d[:])
            o1x128 = const.tile([1, 128], f32r, tag="o1x128")
            nc.sync.dma_start(o1x128[:], o1x128_d[:])
            taba_t = const.tile([128, T], f32, tag="taba")
            tabb_t = const.tile([128, T], f32, tag="tabb")

            # persistent bf16 operands
            qt = [p_big.tile([128, T], bf16, tag=f"qt{h}", name=f"qt{h}")
                  for h in range(QH)]
            ktl = [p_big.tile([128, T], bf16, tag=f"kt{g2}", name=f"kt{g2}")
                   for g2 in range(KVH)]
            vext = [p_big.tile([128, KTOK, HD + 1], bf16, tag=f"vx{g2}",
                               name=f"vx{g2}") for g2 in range(KVH)]
            ao = [p_big.tile([128, T], bf16, tag=f"ao{f}", name=f"ao{f}")
                  for f in range(NF)]
            wot_t = [p_big.tile([128, D], bf16, tag=f"wo{f}", name=f"wo{f}")
                     for f in range(NF)]

            xt_pool = ctx.enter_context(tc.tile_pool(name="xtp", bufs=1))
            xt_t = xt_pool.tile([128, KT, T], bf16, tag="xt")
            nc.sync.dma_start(xt_t[:, 0:6, :], xt_d[:, 0:6, :])
            for c in range(1, 4):
                nc.gpsimd.dma_start(xt_t[:, 6 * c:6 * c + 6, :],
                                    xt_d[:, 6 * c:6 * c + 6, :])

            # zero the rope pad rows (48:64, 112:128 stay zero; rope only
            # writes 0:48 and 64:112)
            for h in range(QH):
                nc.gpsimd.memset(qt[h][:], 0.0)
            for g2 in range(KVH):
                nc.gpsimd.memset(ktl[g2][:], 0.0)
                nc.gpsimd.dma_start(vext[g2][:, :, HD:HD + 1], ocol_d[:])

            # input / weight streaming (rope tables issue in-loop, after
            # the first projection weight clears the sync queue)
            wqk_pool = ctx.enter_context(tc.tile_pool(name="wqk", bufs=2))
            wv_pool = ctx.enter_context(tc.tile_pool(name="wv", bufs=1))

            # SBUF working pools
            sq_pool = ctx.enter_context(tc.tile_pool(name="sqp", bufs=2))
            psc_pool = ctx.enter_context(tc.tile_pool(name="pscp", bufs=3))
            row_pool = ctx.enter_context(tc.tile_pool(name="rowp", bufs=2))
            bcs_pool = ctx.enter_context(tc.tile_pool(name="bcsp", bufs=2))
            qn_pool = ctx.enter_context(tc.tile_pool(name="qnp", bufs=2))
            qsh_pool = ctx.enter_context(tc.tile_pool(name="qshp", bufs=2))
            tm_pool = ctx.enter_context(tc.tile_pool(name="tmp", bufs=4))
            vt_pool = ctx.enter_context(tc.tile_pool(name="vtp", bufs=2))
            pt_pool = ctx.enter_context(tc.tile_pool(name="ptp", bufs=8))
            fin_pool = ctx.enter_context(tc.tile_pool(name="finp", bufs=2))
            ob_pool = ctx.enter_context(tc.tile_pool(name="obp", bufs=2))

            # PSUM: 8 banks of [128, 2KB].  Every buf is a FULL exclusive
            # bank (sub-bank sharing risks concurrent PE-write + engine-read
            # of one bank, which is fatal on HW; full-bank bufs make all
            # reuse WAR-gated by the tile framework).
            ps_pool = ctx.enter_context(
                tc.tile_pool(name="psp", bufs=2, space="PSUM"))   # 2 banks
            ssq_pool = ctx.enter_context(
                tc.tile_pool(name="ssqp", bufs=1, space="PSUM"))  # 1 bank
            bcp_pool = ctx.enter_context(
                tc.tile_pool(name="bcp", bufs=1, space="PSUM"))   # 1 bank
            scp_pool = ctx.enter_context(
                tc.tile_pool(name="scp", bufs=2, space="PSUM"))   # 2 banks
            pop_pool = ctx.enter_context(
                tc.tile_pool(name="pop", bufs=2, space="PSUM"))   # 2 banks

            outs = [("k", 0), ("v", 0), ("k", 1), ("v", 1)] + \
                   [("q", h) for h in range(QH)]

            # ---- per-sub-output stages -------------------------------------
            def stage0(kind, idx, half, ps):
                """scalar square + vector copy of the projection result."""
                if kind == "v":
                    vt = vt_pool.tile([HD, HALF], bf16, tag="vt")
                    nc.scalar.copy(vt[:], ps[0:HD, :])
                    return (vt,)
                sq = sq_pool.tile([128, HALF], f32r, tag="sq")
                nc.scalar.square(sq[:], ps[:])
                psc = psc_pool.tile([128, HALF], f32r, tag="psc")
                nc.vector.tensor_copy(psc[:], ps[:])
                return (sq, psc)

            def stage1(kind, idx, half, st0):
                """tensor: ssq matmul (q/k) or v transposes; scalar: ln/exp."""
                if kind == "v":
                    (vt,) = st0
                    for c in range(HALF // 128):
                        tpt = scp_pool.tile([128, 1024], bf16, tag="sc",
                                            name="tpt")
                        nc.tensor.transpose(
                            tpt[:, 0:HD], vt[:, c * 128:(c + 1) * 128],
                            ident[0:HD, 0:HD])
                        kidx = half * (HALF // 128) + c
                        nc.scalar.copy(vext[idx][:, kidx, 0:HD],
                                       tpt[:, 0:HD])
                    return None
                sq, psc = st0
                ssq = ssq_pool.tile([1, HALF], f32, tag="ssq")
                nc.tensor.matmul(ssq[:], ones128[:], sq[:],
                                 start=True, stop=True)
                rms = row_pool.tile([1, HALF], f32r, tag="rms")
                nc.scalar.activation(rms[:], ssq[:], AF.Sqrt,
                                     bias=eps_t[:], scale=1.0 / HD)
                return (psc, rms)

            def stage2(kind, idx, half, st1):
                """tensor: rms broadcast; vector: reciprocal+normalize+rope."""
                psc, rms = st1
                tsl = slice(half * HALF, (half + 1) * HALF)
                bc = bcp_pool.tile([128, HALF], f32, tag="bc")
                nc.tensor.matmul(bc[:], o1x128[:], rms[:],
                                 start=True, stop=True)
                bcs = bcs_pool.tile([128, HALF], f32, tag="bcs")
                nc.vector.reciprocal_approx_fast(bcs[:], bc[:])
                qn = qn_pool.tile([128, HALF], f32, tag="qn")
                nc.vector.tensor_mul(qn[:], psc[:], bcs[:])
                dst = qt[idx] if kind == "q" else ktl[idx]
                # swap halves (DMA partition shuffle) for partition-aligned rope
                qsh = qsh_pool.tile([128, HALF], f32, tag="qsh")
                nc.gpsimd.dma_start(qsh[0:64, :], qn[64:128, :])
                nc.gpsimd.dma_start(qsh[64:128, :], qn[0:64, :])
                tm1 = tm_pool.tile([128, HALF], f32, tag="tm")
                tm2 = tm_pool.tile([128, HALF], f32, tag="tm")
                nc.vector.tensor_mul(tm1[0:112, :], qn[0:112, :],
                                     taba_t[0:112, tsl])
                nc.vector.tensor_mul(tm2[0:112, :], qsh[0:112, :],
                                     tabb_t[0:112, tsl])
                nc.vector.tensor_add(dst[0:112, tsl], tm1[0:112, :],
                                     tm2[0:112, :])

            # ---- attention -------------------------------------------------
            fin_q = []

            def finalize(h, qi, po):
                """vector: 1/denom + scale; tensor: transpose; pack into ao."""
                r0 = (HD * h) % 128
                f0 = (HD * h) // 128
                n0 = min(HD, 128 - r0)
                qsl = slice(qi * 128, (qi + 1) * 128)
                rinv2 = fin_pool.tile([128, 1], f32, tag="rv2")
                nc.vector.reciprocal_approx_fast(rinv2[:], po[:, HD:HD + 1])
                at_q = fin_pool.tile([128, HD], bf16, tag="atq")
                nc.vector.tensor_scalar_mul(at_q[:], po[:, 0:HD], rinv2[:])

                def emit():
                    tpa = scp_pool.tile([128, 1024], bf16, tag="sc",
                                        name="tpa")
                    nc.tensor.transpose(tpa[0:n0, 0:128], at_q[:, 0:n0],
                                        ident[:, :])
                    stg = fin_pool.tile([128, 128], bf16, tag="stg")
                    nc.vector.tensor_copy(stg[0:n0, :], tpa[0:n0, 0:128])
                    nc.sync.dma_start(ao[f0][r0:r0 + n0, qsl],
                                      stg[0:n0, :])
                    if n0 < HD:
                        n1 = HD - n0
                        tpb = scp_pool.tile([128, 1024], bf16, tag="sc",
                                            name="tpb")
                        nc.tensor.transpose(tpb[0:n1, 0:128], at_q[:, n0:HD],
                                            ident[:, :])
                        stg2 = fin_pool.tile([128, 128], bf16, tag="stg")
                        nc.vector.tensor_copy(stg2[0:n1, :], tpb[0:n1, 0:128])
                        nc.sync.dma_start(ao[f0 + 1][0:n1, qsl],
                                          stg2[0:n1, :])
                fin_q.append(emit)

            def drain_finalize():
                while fin_q:
                    fin_q.pop(0)()

            def attention_chunk(h, qi):
                g2 = h // (QH // KVH)
                qsl = slice(qi * 128, (qi + 1) * 128)
                nk = qi + 1
                # scores in groups of 4 k-tiles per PSUM bank: one 512-wide
                # exp per bank instead of four 128-wide ones
                pts = []
                for b0 in range(0, nk, 4):
                    nb = min(4, nk - b0)
                    sc_t = scp_pool.tile([128, 4, 128], f32,
                                         tag="sc", name="sc_t")
                    for j in range(nb):
                        kt2 = b0 + j
                        nc.tensor.matmul(
                            sc_t[:, j, :],
                            ktl[g2][:, kt2 * 128:(kt2 + 1) * 128],
                            qt[h][:, qsl], start=True, stop=True)
                    pt = pt_pool.tile([128, 4, 128], bf16, tag="pt")
                    nc.scalar.activation(pt[:, 0:nb, :], sc_t[:, 0:nb, :],
                                         AF.Exp, scale=SCALE)
                    pts.append(pt)
                drain_finalize()
                jd = qi % 4
                nc.vector.tensor_mul(pts[-1][:, jd, :], pts[-1][:, jd, :],
                                     tri[:])
                po = pop_pool.tile([128, 512], f32, tag="po")
                for kt2 in range(nk):
                    nc.tensor.matmul(po[:, 0:HD + 1], pts[kt2 // 4][:, kt2 % 4, :],
                                     vext[g2][:, kt2, :],
                                     start=(kt2 == 0), stop=(kt2 == qi))
                finalize(h, qi, po)

            def attention(h):
                for qi in range(KTOK):
                    attention_chunk(h, qi)

            # ---- phase 1+2 interleaved issue loop --------------------------
            SUBS = [(kind, idx, half) for kind, idx in outs
                    for half in range(2)]
            st0q, st1q = [], []
            w_cur = [None]

            for s, (kind, idx, half) in enumerate(SUBS):
                if s == 1:
                    nc.sync.dma_start(taba_t[:], taba_d[:])
                    nc.sync.dma_start(tabb_t[:], tabb_d[:])
                if s == 12:
                    for f in range(NF):
                        nc.sync.dma_start(wot_t[f][:], wot_d[f])
                if half == 0:
                    if kind == "q":
                        w_t = wqk_pool.tile([128, KT, 128], bf16, tag="w")
                        nc.sync.dma_start(w_t[:], wqt_d[idx])
                        mdim = 128
                    elif kind == "k":
                        w_t = wqk_pool.tile([128, KT, 128], bf16, tag="w")
                        nc.sync.dma_start(w_t[:], wkt_d[idx])
                        mdim = 128
                    else:
                        w_t = wv_pool.tile([128, KT, HD], bf16, tag="wv")
                        nc.sync.dma_start(w_t[:], wvt_d[idx])
                        mdim = HD
                    w_cur[0] = (w_t, mdim)
                w_t, mdim = w_cur[0]
                tsl = slice(half * HALF, (half + 1) * HALF)
                ps = ps_pool.tile([128, HALF], f32, tag="ps")
                for kt in range(KT):
                    nc.tensor.matmul(ps[0:mdim, :], w_t[:, kt, :],
                                     xt_t[:, kt, tsl],
                                     start=(kt == 0), stop=(kt == KT - 1))
                # deferred norm work between projection groups
                if st1q:
                    k_, i_, h_, p_ = st1q.pop(0)
                    stage2(k_, i_, h_, p_)
                if st0q:
                    k_, i_, h_, p_ = st0q.pop(0)
                    r = stage1(k_, i_, h_, p_)
                    if r is not None:
                        st1q.append((k_, i_, h_, r))
                st0q.append((kind, idx, half, stage0(kind, idx, half, ps)))

            # flush pipeline, then remaining attention heads
            for _ in range(3):
                if st1q:
                    k_, i_, h_, p_ = st1q.pop(0)
                    stage2(k_, i_, h_, p_)
                if st0q:
                    k_, i_, h_, p_ = st0q.pop(0)
                    r = stage1(k_, i_, h_, p_)
                    if r is not None:
                        st1q.append((k_, i_, h_, r))
            def phase3_col(i):
                isl = slice(i * 128, (i + 1) * 128)
                for jh in range(2):
                    ob = ob_pool.tile([128, D // 2], bf16, tag="ob")
                    for j in range(3):
                        jsl = slice((3 * jh + j) * 512, (3 * jh + j + 1) * 512)
                        ps3 = ps_pool.tile([128, 512], f32, tag="ps")
                        for f in range(NF):
                            nc.tensor.matmul(ps3[:], ao[f][:, isl],
                                             wot_t[f][:, jsl],
                                             start=(f == 0),
                                             stop=(f == NF - 1))
                        nc.scalar.copy(ob[:, j * 512:(j + 1) * 512], ps3[:])
                    nc.sync.dma_start(
                        out_d[isl, jh * (D // 2):(jh + 1) * (D // 2)], ob[:])

            # qi-major attention: all heads compute q-column qi, then the
            # o_proj for column qi-1 interleaves behind it.  Attention's exp
            # work (scalar) hides under o_proj matmuls, and phase 1 ran with
            # zero activation-table switches.
            for qi in range(KTOK):
                for h in range(QH):
                    attention_chunk(h, qi)
                if qi >= 1:
                    phase3_col(qi - 1)
            drain_finalize()
            phase3_col(KTOK - 1)

    nc.compile()
    return nc


def get_nc():
    if "nc" not in _BUILD_CACHE:
        _BUILD_CACHE["nc"] = _build_nc()
    return _BUILD_CACHE["nc"]


def _permpad_rows(w96):
    """(96, N) head rows -> (128, N): evens at 0:48, odds at 64:112, pad 0."""
    out = np.zeros((128, w96.shape[1]), np.float32)
    out[0:48] = w96[0::2]
    out[64:112] = w96[1::2]
    return out


def _lhsT_tiles(wT, m):
    """(D, m) -> (128, KT, m) lhsT tile layout (contraction on partitions)."""
    return np.ascontiguousarray(
        wT.reshape(KT, 128, m).transpose(1, 0, 2)).astype(np.float32)


def prepare_in_maps(x, wq, wk, wv, wo, q_norm_w, k_norm_w, cos, sin):
    import ml_dtypes
    bf16 = ml_dtypes.bfloat16

    x = np.asarray(x, np.float32)
    wq = np.asarray(wq, np.float32) * np.tile(
        np.asarray(q_norm_w, np.float32), NH)[:, None]
    wk = np.asarray(wk, np.float32) * np.tile(
        np.asarray(k_norm_w, np.float32), NKV)[:, None]
    wv = np.asarray(wv, np.float32)
    wo = np.asarray(wo, np.float32)
    cos = np.asarray(cos, np.float32)
    sin = np.asarray(sin, np.float32)

    # fused-rope tables: dst = qn*P + qsh*Q (P=cos rows, Q=sin rows with
    # the even-row sign folded in); pad rows stay zero
    taba = np.zeros((128, T), np.float32)
    tabb = np.zeros((128, T), np.float32)
    taba[0:48] = cos[:, 0::2].T
    taba[64:112] = cos[:, 1::2].T
    tabb[0:48] = -sin[:, 0::2].T
    tabb[64:112] = sin[:, 1::2].T

    xts = []
    for b in range(B):
        xT = np.ascontiguousarray(x[b].T)  # (D, T)
        xts.append(np.ascontiguousarray(
            xT.reshape(KT, 128, T).transpose(1, 0, 2)).astype(bf16))

    tri = np.triu(np.ones((128, 128), np.float32)).astype(bf16)
    identm = np.eye(128, dtype=np.float32).astype(bf16)

    in_maps = []
    for c in range(NCORES):
        b, g = divmod(c, G)
        wqt = np.stack([
            _lhsT_tiles(_permpad_rows(
                wq[(g * QH + i) * HD:(g * QH + i + 1) * HD]).T, 128)
            for i in range(QH)]).astype(bf16)
        wkt = np.stack([
            _lhsT_tiles(_permpad_rows(
                wk[(g * KVH + i) * HD:(g * KVH + i + 1) * HD]).T, 128)
            for i in range(KVH)]).astype(bf16)
        wvt = np.stack([
            _lhsT_tiles(np.ascontiguousarray(
                wv[(g * KVH + i) * HD:(g * KVH + i + 1) * HD].T), HD)
            for i in range(KVH)]).astype(bf16)
        wo_shT = np.ascontiguousarray(
            wo[:, g * QH * HD:(g + 1) * QH * HD].T)  # (768, D)
        wot = np.ascontiguousarray(
            wo_shT.reshape(NF, 128, D)).astype(bf16)
        in_maps.append({
            "xt": xts[b], "wqt": wqt, "wkt": wkt, "wvt": wvt, "wot": wot,
            "taba": taba, "tabb": tabb,
            "o128": np.ones((128, 1), np.float32),
            "o1x128": np.ones((1, 128), np.float32),
            "ocol": np.ones((128, KTOK), np.float32).astype(bf16),
            "ident": identm, "tri": tri,
        })
    return in_maps


def kernel(**inputs):
    from concourse import bass_utils

    nc = get_nc()
    in_maps = prepare_in_maps(
        inputs["x"], inputs["wq"], inputs["wk"], inputs["wv"], inputs["wo"],
        inputs["q_norm_w"], inputs["k_norm_w"], inputs["cos"], inputs["sin"])
    trace = bool(int(os.environ.get("BASS_KERNEL_TRACE", "0")))
    tmpdir = os.environ.get("BASS_KERNEL_TMPDIR") or None
    res = bass_utils.run_bass_kernel_spmd(
        nc, in_maps, core_ids=list(range(NCORES)), trace=trace, tmpdir=tmpdir)
    _BUILD_CACHE["last_result"] = res
    partials = [np.asarray(r["out"]).astype(np.float32) for r in res.results]
    out = np.empty((B, T, D), np.float32)
    for b in range(B):
        out[b] = np.sum(np.stack(partials[b * G:(b + 1) * G]), axis=0,
                        dtype=np.float64).astype(np.float32)
    return out


# revision 49
# speedup vs baseline: 1.0173x; 1.0058x over previous
"""Grouped-Query Attention block (RMSNorm + RoPE + causal GQA + o_proj) on 8 trn2 NeuronCores.

Sharding: data-parallel over batch (2) x tensor-parallel over kv-head groups (4).
Core c = b*4 + g handles batch b, kv heads {2g, 2g+1}, q heads {8g..8g+7}.
Each core computes a partial o_proj output (T, D) over its 768 head-dims;
host sums the 4 group partials per batch.

v2 layout/schedule (from trace analysis of v1):
  * All matmul operands bf16 (fp32 PSUM accumulate): full PE rate, fast
    weight load, half the DMA bytes.  Norm/rope math stays fp32.
  * RMSNorm 1/rms computed as exp(-0.5*ln(ms+eps)) on the scalar engine:
    ln/exp/copy/square all live in one activation table (no table thrash
    with attention's exp); the broadcast over head rows is a ones-outer-
    product matmul.  No single-lane [1,N] vector reciprocals (4us each in
    v1, 145us total, on the critical path).
  * Per-head even/odd RoPE interleave folded into wq/wk rows (evens 0:48,
    odds 64:112, zero pad), q/k norm weights folded into wq/wk.
  * Attention: scores transposed (k on partitions, q free) in 128-wide
    q chunks with exact tile-level causality; P@V uses P as stationary so
    the output is [q, hd+1] with the softmax denominator as a per-partition
    scalar (vector reciprocal_approx_fast on [128,1]); diagonal masked by a
    constant triangular multiply on the vector engine.
  * Attention outputs transposed back and packed into 6 dense 128-row
    feature tiles so o_proj is 6 (not 8) matmuls per output tile.
  * Issue order software-pipelined: the norm matmuls of output j are issued
    after the projection group of output j+1; attention for head h is
    interleaved between later projection groups; attention finalize for
    chunk qi is issued after the score matmuls of chunk qi+1.
"""

import os
import sys

import numpy as np

sys.path.insert(0, "/opt/trn_rl_repo")

B, T, D = 2, 1024, 3072
NH, NKV, HD = 32, 8, 96
G = 4                 # tensor-parallel groups
QH = NH // G          # q heads per core (8)
KVH = NKV // G        # kv heads per core (2)
NCORES = 8
EPS = 1e-6
SCALE = 1.0 / float(np.sqrt(HD))
KT = D // 128         # 24 contraction tiles over d_model
KTOK = T // 128       # 8 token tiles
NF = QH * HD // 128   # 6 packed feature tiles for o_proj
HALF = T // 2         # 512

_BUILD_CACHE = {}


def _build_nc():
    from contextlib import ExitStack
    from concourse import bacc, tile, mybir

    f32 = mybir.dt.float32
    f32r = mybir.dt.float32r
    bf16 = mybir.dt.bfloat16
    AF = mybir.ActivationFunctionType

    nc = bacc.Bacc("TRN2", target_bir_lowering=False, debug=False,
                   num_devices=NCORES)

    xt_d = nc.dram_tensor("xt", (128, KT, T), bf16, kind="ExternalInput").ap()
    wqt_d = nc.dram_tensor("wqt", (QH, 128, KT, 128), bf16, kind="ExternalInput").ap()
    wkt_d = nc.dram_tensor("wkt", (KVH, 128, KT, 128), bf16, kind="ExternalInput").ap()
    wvt_d = nc.dram_tensor("wvt", (KVH, 128, KT, HD), bf16, kind="ExternalInput").ap()
    wot_d = nc.dram_tensor("wot", (NF, 128, D), bf16, kind="ExternalInput").ap()
    taba_d = nc.dram_tensor("taba", (128, T), f32, kind="ExternalInput").ap()
    tabb_d = nc.dram_tensor("tabb", (128, T), f32, kind="ExternalInput").ap()
    o128_d = nc.dram_tensor("o128", (128, 1), f32r, kind="ExternalInput").ap()
    o1x128_d = nc.dram_tensor("o1x128", (1, 128), f32r, kind="ExternalInput").ap()
    ocol_d = nc.dram_tensor("ocol", (128, KTOK), bf16, kind="ExternalInput").ap()
    ident_d = nc.dram_tensor("ident", (128, 128), bf16, kind="ExternalInput").ap()
    tri_d = nc.dram_tensor("tri", (128, 128), bf16, kind="ExternalInput").ap()
    out_d = nc.dram_tensor("out", (T, D), bf16, kind="ExternalOutput").ap()

    with tile.TileContext(nc) as tc:
        with nc.allow_low_precision(reason="bf16 matmuls, fp32 accumulate"), \
             ExitStack() as ctx:
            const = ctx.enter_context(tc.tile_pool(name="const", bufs=1))
            p_big = ctx.enter_context(tc.tile_pool(name="p_big", bufs=1))

            ident = const.tile([128, 128], bf16, tag="ident")
            nc.sync.dma_start(ident[:], ident_d[:])
            tri = const.tile([128, 128], bf16, tag="tri")
            nc.sync.dma_start(tri[:], tri_d[:])
            eps_t = const.tile([1, 1], f32, tag="eps")
            nc.vector.memset(eps_t[:], EPS)
            ones128 = const.tile([128, 1], f32r, tag="ones128")
            nc.sync.dma_start(ones128[:], o128_d[:])
            o1x128 = const.tile([1, 128], f32r, tag="o1x128")
            nc.sync.dma_start(o1x128[:], o1x128_d[:])
            taba_t = const.tile([128, T], f32, tag="taba")
            tabb_t = const.tile([128, T], f32, tag="tabb")

            # persistent bf16 operands
            qt = [p_big.tile([128, T], bf16, tag=f"qt{h}", name=f"qt{h}")
                  for h in range(QH)]
            ktl = [p_big.tile([128, T], bf16, tag=f"kt{g2}", name=f"kt{g2}")
                   for g2 in range(KVH)]
            vext = [p_big.tile([128, KTOK, HD + 1], bf16, tag=f"vx{g2}",
                               name=f"vx{g2}") for g2 in range(KVH)]
            ao = [p_big.tile([128, T], bf16, tag=f"ao{f}", name=f"ao{f}")
                  for f in range(NF)]
            wot_t = [p_big.tile([128, D], bf16, tag=f"wo{f}", name=f"wo{f}")
                     for f in range(NF)]

            xt_pool = ctx.enter_context(tc.tile_pool(name="xtp", bufs=1))
            xt_t = xt_pool.tile([128, KT, T], bf16, tag="xt")
            wqk_pool0 = ctx.enter_context(tc.tile_pool(name="wqk", bufs=2))
            w_first = wqk_pool0.tile([128, KT, 128], bf16, tag="w")
            nc.sync.dma_start(w_first[:, 0:12, :], wkt_d[0][:, 0:12, :])
            nc.sync.dma_start(xt_t[:, 0:3, :], xt_d[:, 0:3, :])
            nc.sync.dma_start(w_first[:, 12:24, :], wkt_d[0][:, 12:24, :])
            nc.sync.dma_start(xt_t[:, 3:6, :], xt_d[:, 3:6, :])
            for c in range(1, 4):
                nc.gpsimd.dma_start(xt_t[:, 6 * c:6 * c + 6, :],
                                    xt_d[:, 6 * c:6 * c + 6, :])

            # zero the rope pad rows (48:64, 112:128 stay zero; rope only
            # writes 0:48 and 64:112)
            for h in range(QH):
                nc.gpsimd.memset(qt[h][:], 0.0)
            for g2 in range(KVH):
                nc.gpsimd.memset(ktl[g2][:], 0.0)
                nc.gpsimd.dma_start(vext[g2][:, :, HD:HD + 1], ocol_d[:])

            # input / weight streaming (rope tables issue in-loop, after
            # the first projection weight clears the sync queue)
            wqk_pool = wqk_pool0
            wv_pool = ctx.enter_context(tc.tile_pool(name="wv", bufs=1))

            # SBUF working pools
            sq_pool = ctx.enter_context(tc.tile_pool(name="sqp", bufs=2))
            psc_pool = ctx.enter_context(tc.tile_pool(name="pscp", bufs=3))
            row_pool = ctx.enter_context(tc.tile_pool(name="rowp", bufs=2))
            bcs_pool = ctx.enter_context(tc.tile_pool(name="bcsp", bufs=2))
            qn_pool = ctx.enter_context(tc.tile_pool(name="qnp", bufs=2))
            qsh_pool = ctx.enter_context(tc.tile_pool(name="qshp", bufs=2))
            tm_pool = ctx.enter_context(tc.tile_pool(name="tmp", bufs=4))
            vt_pool = ctx.enter_context(tc.tile_pool(name="vtp", bufs=2))
            pt_pool = ctx.enter_context(tc.tile_pool(name="ptp", bufs=8))
            fin_pool = ctx.enter_context(tc.tile_pool(name="finp", bufs=2))
            ob_pool = ctx.enter_context(tc.tile_pool(name="obp", bufs=2))

            # PSUM: 8 banks of [128, 2KB].  Every buf is a FULL exclusive
            # bank (sub-bank sharing risks concurrent PE-write + engine-read
            # of one bank, which is fatal on HW; full-bank bufs make all
            # reuse WAR-gated by the tile framework).
            ps_pool = ctx.enter_context(
                tc.tile_pool(name="psp", bufs=2, space="PSUM"))   # 2 banks
            ssq_pool = ctx.enter_context(
                tc.tile_pool(name="ssqp", bufs=1, space="PSUM"))  # 1 bank
            bcp_pool = ctx.enter_context(
                tc.tile_pool(name="bcp", bufs=1, space="PSUM"))   # 1 bank
            scp_pool = ctx.enter_context(
                tc.tile_pool(name="scp", bufs=2, space="PSUM"))   # 2 banks
            pop_pool = ctx.enter_context(
                tc.tile_pool(name="pop", bufs=2, space="PSUM"))   # 2 banks

            outs = [("k", 0), ("v", 0), ("k", 1), ("v", 1)] + \
                   [("q", h) for h in range(QH)]

            # ---- per-sub-output stages -------------------------------------
            def stage0(kind, idx, half, ps):
                """scalar square + vector copy of the projection result."""
                if kind == "v":
                    vt = vt_pool.tile([HD, HALF], bf16, tag="vt")
                    nc.scalar.copy(vt[:], ps[0:HD, :])
                    return (vt,)
                sq = sq_pool.tile([128, HALF], f32r, tag="sq")
                nc.scalar.square(sq[:], ps[:])
                psc = psc_pool.tile([128, HALF], f32r, tag="psc")
                nc.vector.tensor_copy(psc[:], ps[:])
                return (sq, psc)

            def stage1(kind, idx, half, st0):
                """tensor: ssq matmul (q/k) or v transposes; scalar: ln/exp."""
                if kind == "v":
                    (vt,) = st0
                    for c in range(HALF // 128):
                        tpt = scp_pool.tile([128, 1024], bf16, tag="sc",
                                            name="tpt")
                        nc.tensor.transpose(
                            tpt[:, 0:HD], vt[:, c * 128:(c + 1) * 128],
                            ident[0:HD, 0:HD])
                        kidx = half * (HALF // 128) + c
                        nc.scalar.copy(vext[idx][:, kidx, 0:HD],
                                       tpt[:, 0:HD])
                    return None
                sq, psc = st0
                ssq = ssq_pool.tile([1, HALF], f32, tag="ssq")
                nc.tensor.matmul(ssq[:], ones128[:], sq[:],
                                 start=True, stop=True)
                rms = row_pool.tile([1, HALF], f32r, tag="rms")
                nc.scalar.activation(rms[:], ssq[:], AF.Sqrt,
                                     bias=eps_t[:], scale=1.0 / HD)
                return (psc, rms)

            def stage2(kind, idx, half, st1):
                """tensor: rms broadcast; vector: reciprocal+normalize+rope."""
                psc, rms = st1
                tsl = slice(half * HALF, (half + 1) * HALF)
                bc = bcp_pool.tile([128, HALF], f32, tag="bc")
                nc.tensor.matmul(bc[:], o1x128[:], rms[:],
                                 start=True, stop=True)
                bcs = bcs_pool.tile([128, HALF], f32, tag="bcs")
                nc.vector.reciprocal_approx_fast(bcs[:], bc[:])
                qn = qn_pool.tile([128, HALF], f32, tag="qn")
                nc.vector.tensor_mul(qn[:], psc[:], bcs[:])
                dst = qt[idx] if kind == "q" else ktl[idx]
                # swap halves (DMA partition shuffle) for partition-aligned rope
                qsh = qsh_pool.tile([128, HALF], f32, tag="qsh")
                nc.gpsimd.dma_start(qsh[0:64, :], qn[64:128, :])
                nc.gpsimd.dma_start(qsh[64:128, :], qn[0:64, :])
                tm1 = tm_pool.tile([128, HALF], f32, tag="tm")
                tm2 = tm_pool.tile([128, HALF], f32, tag="tm")
                nc.vector.tensor_mul(tm1[0:112, :], qn[0:112, :],
                                     taba_t[0:112, tsl])
                nc.vector.tensor_mul(tm2[0:112, :], qsh[0:112, :],
                                     tabb_t[0:112, tsl])
                nc.vector.tensor_add(dst[0:112, tsl], tm1[0:112, :],
                                     tm2[0:112, :])

            # ---- attention -------------------------------------------------
            fin_q = []

            def finalize(h, qi, po):
                """vector: 1/denom + scale; tensor: transpose; pack into ao."""
                r0 = (HD * h) % 128
                f0 = (HD * h) // 128
                n0 = min(HD, 128 - r0)
                qsl = slice(qi * 128, (qi + 1) * 128)
                rinv2 = fin_pool.tile([128, 1], f32, tag="rv2")
                nc.vector.reciprocal_approx_fast(rinv2[:], po[:, HD:HD + 1])
                at_q = fin_pool.tile([128, HD], bf16, tag="atq")
                nc.vector.tensor_scalar_mul(at_q[:], po[:, 0:HD], rinv2[:])

                def emit():
                    tpa = scp_pool.tile([128, 1024], bf16, tag="sc",
                                        name="tpa")
                    nc.tensor.transpose(tpa[0:n0, 0:128], at_q[:, 0:n0],
                                        ident[:, :])
                    stg = fin_pool.tile([128, 128], bf16, tag="stg")
                    nc.vector.tensor_copy(stg[0:n0, :], tpa[0:n0, 0:128])
                    nc.sync.dma_start(ao[f0][r0:r0 + n0, qsl],
                                      stg[0:n0, :])
                    if n0 < HD:
                        n1 = HD - n0
                        tpb = scp_pool.tile([128, 1024], bf16, tag="sc",
                                            name="tpb")
                        nc.tensor.transpose(tpb[0:n1, 0:128], at_q[:, n0:HD],
                                            ident[:, :])
                        stg2 = fin_pool.tile([128, 128], bf16, tag="stg")
                        nc.vector.tensor_copy(stg2[0:n1, :], tpb[0:n1, 0:128])
                        nc.sync.dma_start(ao[f0 + 1][0:n1, qsl],
                                          stg2[0:n1, :])
                fin_q.append(emit)

            def drain_finalize():
                while fin_q:
                    fin_q.pop(0)()

            def attention_chunk(h, qi):
                g2 = h // (QH // KVH)
                qsl = slice(qi * 128, (qi + 1) * 128)
                nk = qi + 1
                # scores in groups of 4 k-tiles per PSUM bank: one 512-wide
                # exp per bank instead of four 128-wide ones
                pts = []
                for b0 in range(0, nk, 4):
                    nb = min(4, nk - b0)
                    sc_t = scp_pool.tile([128, 4, 128], f32,
                                         tag="sc", name="sc_t")
                    for j in range(nb):
                        kt2 = b0 + j
                        nc.tensor.matmul(
                            sc_t[:, j, :],
                            ktl[g2][:, kt2 * 128:(kt2 + 1) * 128],
                            qt[h][:, qsl], start=True, stop=True)
                    pt = pt_pool.tile([128, 4, 128], bf16, tag="pt")
                    nc.scalar.activation(pt[:, 0:nb, :], sc_t[:, 0:nb, :],
                                         AF.Exp, scale=SCALE)
                    pts.append(pt)
                drain_finalize()
                jd = qi % 4
                nc.vector.tensor_mul(pts[-1][:, jd, :], pts[-1][:, jd, :],
                                     tri[:])
                po = pop_pool.tile([128, 512], f32, tag="po")
                for kt2 in range(nk):
                    nc.tensor.matmul(po[:, 0:HD + 1], pts[kt2 // 4][:, kt2 % 4, :],
                                     vext[g2][:, kt2, :],
                                     start=(kt2 == 0), stop=(kt2 == qi))
                finalize(h, qi, po)

            def attention(h):
                for qi in range(KTOK):
                    attention_chunk(h, qi)

            # ---- phase 1+2 interleaved issue loop --------------------------
            SUBS = [(kind, idx, half) for kind, idx in outs
                    for half in range(2)]
            st0q, st1q = [], []
            w_cur = [None]

            for s, (kind, idx, half) in enumerate(SUBS):
                if s == 1:
                    nc.sync.dma_start(taba_t[:], taba_d[:])
                    nc.sync.dma_start(tabb_t[:], tabb_d[:])
                if s == 12:
                    for f in range(NF):
                        nc.sync.dma_start(wot_t[f][:], wot_d[f])
                if half == 0:
                    if kind == "q":
                        w_t = wqk_pool.tile([128, KT, 128], bf16, tag="w")
                        nc.sync.dma_start(w_t[:], wqt_d[idx])
                        mdim = 128
                    elif kind == "k" and idx == 0:
                        w_t = w_first
                        mdim = 128
                    elif kind == "k":
                        w_t = wqk_pool.tile([128, KT, 128], bf16, tag="w")
                        nc.sync.dma_start(w_t[:], wkt_d[idx])
                        mdim = 128
                    else:
                        w_t = wv_pool.tile([128, KT, HD], bf16, tag="wv")
                        nc.sync.dma_start(w_t[:], wvt_d[idx])
                        mdim = HD
                    w_cur[0] = (w_t, mdim)
                w_t, mdim = w_cur[0]
                tsl = slice(half * HALF, (half + 1) * HALF)
                ps = ps_pool.tile([128, HALF], f32, tag="ps")
                for kt in range(KT):
                    nc.tensor.matmul(ps[0:mdim, :], w_t[:, kt, :],
                                     xt_t[:, kt, tsl],
                                     start=(kt == 0), stop=(kt == KT - 1))
                # deferred norm work between projection groups
                if st1q:
                    k_, i_, h_, p_ = st1q.pop(0)
                    stage2(k_, i_, h_, p_)
                if st0q:
                    k_, i_, h_, p_ = st0q.pop(0)
                    r = stage1(k_, i_, h_, p_)
                    if r is not None:
                        st1q.append((k_, i_, h_, r))
                st0q.append((kind, idx, half, stage0(kind, idx, half, ps)))

            # flush pipeline, then remaining attention heads
            for _ in range(3):
                if st1q:
                    k_, i_, h_, p_ = st1q.pop(0)
                    stage2(k_, i_, h_, p_)
                if st0q:
                    k_, i_, h_, p_ = st0q.pop(0)
                    r = stage1(k_, i_, h_, p_)
                    if r is not None:
                        st1q.append((k_, i_, h_, r))
            def phase3_col(i):
                isl = slice(i * 128, (i + 1) * 128)
                for jh in range(2):
                    ob = ob_pool.tile([128, D // 2], bf16, tag="ob")
                    for j in range(3):
                        jsl = slice((3 * jh + j) * 512, (3 * jh + j + 1) * 512)
                        ps3 = ps_pool.tile([128, 512], f32, tag="ps")
                        for f in range(NF):
                            nc.tensor.matmul(ps3[:], ao[f][:, isl],
                                             wot_t[f][:, jsl],
                                             start=(f == 0),
                                             stop=(f == NF - 1))
                        nc.scalar.copy(ob[:, j * 512:(j + 1) * 512], ps3[:])
                    nc.sync.dma_start(
                        out_d[isl, jh * (D // 2):(jh + 1) * (D // 2)], ob[:])

            # qi-major attention: all heads compute q-column qi, then the
            # o_proj for column qi-1 interleaves behind it.  Attention's exp
            # work (scalar) hides under o_proj matmuls, and phase 1 ran with
            # zero activation-table switches.
            for qi in range(KTOK):
                for h in range(QH):
                    attention_chunk(h, qi)
                if qi >= 1:
                    phase3_col(qi - 1)
            drain_finalize()
            phase3_col(KTOK - 1)

    nc.compile()
    return nc


def get_nc():
    if "nc" not in _BUILD_CACHE:
        _BUILD_CACHE["nc"] = _build_nc()
    return _BUILD_CACHE["nc"]


def _permpad_rows(w96):
    """(96, N) head rows -> (128, N): evens at 0:48, odds at 64:112, pad 0."""
    out = np.zeros((128, w96.shape[1]), np.float32)
    out[0:48] = w96[0::2]
    out[64:112] = w96[1::2]
    return out


def _lhsT_tiles(wT, m):
    """(D, m) -> (128, KT, m) lhsT tile layout (contraction on partitions)."""
    return np.ascontiguousarray(
        wT.reshape(KT, 128, m).transpose(1, 0, 2)).astype(np.float32)


def prepare_in_maps(x, wq, wk, wv, wo, q_norm_w, k_norm_w, cos, sin):
    import ml_dtypes
    bf16 = ml_dtypes.bfloat16

    x = np.asarray(x, np.float32)
    wq = np.asarray(wq, np.float32) * np.tile(
        np.asarray(q_norm_w, np.float32), NH)[:, None]
    wk = np.asarray(wk, np.float32) * np.tile(
        np.asarray(k_norm_w, np.float32), NKV)[:, None]
    wv = np.asarray(wv, np.float32)
    wo = np.asarray(wo, np.float32)
    cos = np.asarray(cos, np.float32)
    sin = np.asarray(sin, np.float32)

    # fused-rope tables: dst = qn*P + qsh*Q (P=cos rows, Q=sin rows with
    # the even-row sign folded in); pad rows stay zero
    taba = np.zeros((128, T), np.float32)
    tabb = np.zeros((128, T), np.float32)
    taba[0:48] = cos[:, 0::2].T
    taba[64:112] = cos[:, 1::2].T
    tabb[0:48] = -sin[:, 0::2].T
    tabb[64:112] = sin[:, 1::2].T

    xts = []
    for b in range(B):
        xT = np.ascontiguousarray(x[b].T)  # (D, T)
        xts.append(np.ascontiguousarray(
            xT.reshape(KT, 128, T).transpose(1, 0, 2)).astype(bf16))

    tri = np.triu(np.ones((128, 128), np.float32)).astype(bf16)
    identm = np.eye(128, dtype=np.float32).astype(bf16)

    in_maps = []
    for c in range(NCORES):
        b, g = divmod(c, G)
        wqt = np.stack([
            _lhsT_tiles(_permpad_rows(
                wq[(g * QH + i) * HD:(g * QH + i + 1) * HD]).T, 128)
            for i in range(QH)]).astype(bf16)
        wkt = np.stack([
            _lhsT_tiles(_permpad_rows(
                wk[(g * KVH + i) * HD:(g * KVH + i + 1) * HD]).T, 128)
            for i in range(KVH)]).astype(bf16)
        wvt = np.stack([
            _lhsT_tiles(np.ascontiguousarray(
                wv[(g * KVH + i) * HD:(g * KVH + i + 1) * HD].T), HD)
            for i in range(KVH)]).astype(bf16)
        wo_shT = np.ascontiguousarray(
            wo[:, g * QH * HD:(g + 1) * QH * HD].T)  # (768, D)
        wot = np.ascontiguousarray(
            wo_shT.reshape(NF, 128, D)).astype(bf16)
        in_maps.append({
            "xt": xts[b], "wqt": wqt, "wkt": wkt, "wvt": wvt, "wot": wot,
            "taba": taba, "tabb": tabb,
            "o128": np.ones((128, 1), np.float32),
            "o1x128": np.ones((1, 128), np.float32),
            "ocol": np.ones((128, KTOK), np.float32).astype(bf16),
            "ident": identm, "tri": tri,
        })
    return in_maps


def kernel(**inputs):
    from concourse import bass_utils

    nc = get_nc()
    in_maps = prepare_in_maps(
        inputs["x"], inputs["wq"], inputs["wk"], inputs["wv"], inputs["wo"],
        inputs["q_norm_w"], inputs["k_norm_w"], inputs["cos"], inputs["sin"])
    trace = bool(int(os.environ.get("BASS_KERNEL_TRACE", "0")))
    tmpdir = os.environ.get("BASS_KERNEL_TMPDIR") or None
    res = bass_utils.run_bass_kernel_spmd(
        nc, in_maps, core_ids=list(range(NCORES)), trace=trace, tmpdir=tmpdir)
    _BUILD_CACHE["last_result"] = res
    partials = [np.asarray(r["out"]).astype(np.float32) for r in res.results]
    out = np.empty((B, T, D), np.float32)
    for b in range(B):
        out[b] = np.sum(np.stack(partials[b * G:(b + 1) * G]), axis=0,
                        dtype=np.float64).astype(np.float32)
    return out
